# revision 4
# baseline (speedup 1.0000x reference)
"""Trainium2 Bass kernel for nn_CSFlow (RAFT-style correlation pyramid lookup).

Math restructure (exact up to fp16 rounding of stored corr values):
  - corr(q, pos) = <fmap1[:, q], fmap2[:, pos]> / sqrt(D). Pooling the corr
    volume over (i, j) == pooling fmap2 (linearity), so each pyramid level is
    its own matmul against a pooled fmap2.
  - All 81 lookup offsets of one query share the same fractional bilinear
    weights (integer offsets), so the lookup = gather of a 10x10 integer
    window + separable 2-tap blends with per-query weights.
  - The 10x10 window at a per-query position is fetched from an HBM scratch
    copy of that query's corr map with ONE indirect-DMA descriptor per
    (query, level): a contiguous band of 9*S+10 elements (S = inner-axis
    size) starting at the window origin. The data between window rows inside
    the band is simply never read (strided view). Out-of-range taps are
    zeroed exactly via host-precomputed masks folded into the stage-1 blend
    weights; band reads that spill outside a query's map hit neighbouring
    maps / pre-zeroed guard rows, so they are finite and masked.
  - Levels 0-2 store maps x-major (inner = y, size H_l >= 12); level 3
    stores y-major (inner = x, size 20) because H_3 = 6 < 10 would make the
    window view overlap. The host permutes level-3 output channels back.

Engine layout (per query tile): PE runs the corr matmuls back-to-back into
4 psum bufs; psum->sbuf fp16 copies are split across DVE/ACT/Pool; all
scratch-write DMA triggers live on the SP sequencer (cheapest trigger, no
compute to displace); gathers are Pool SWDGE; blends on DVE (stage 1) and
ACT+DVE (stage 2). Output tiles DMA out per tile in query-major layout;
the host does the [q, lvl*81] -> [lvl*81, q] reorder in assemble().

Sharding: 8 cores x 1920 queries (B*H*W = 15360 split contiguously; cores
0-3 handle batch 0, cores 4-7 batch 1). kernel() takes full inputs and
returns the full output; everything device-side runs SPMD on 8 cores.
"""

import numpy as np

import concourse.bass as bass
import concourse.mybir as mybir
import concourse.tile as tile
from concourse import bacc
from concourse.bass_utils import run_bass_kernel_spmd

# problem shape (hardcoded per harness contract)
B, D, H, W = 2, 256, 48, 160
NCORES = 8
QPC = (B * H * W) // NCORES      # 1920 queries per core
P = 128                          # queries per tile (partitions)
NT = QPC // P                    # 15 tiles per core
NLVL = 4
LH = [48, 24, 12, 6]
LW = [160, 80, 40, 20]
LHW = [LH[i] * LW[i] for i in range(NLVL)]           # 7680 1920 480 120
LOFF = [0, 7680, 9600, 10080]                        # col offset in f2 concat
NPOS = 10200
L0N = LHW[0]                                         # 7680
RESTN = NPOS - L0N                                   # 2520
XMAJ = [True, True, True, False]                     # storage orientation
ST = [48, 24, 12, 20]                                # inner-axis size
BAND = [9 * s + 10 for s in ST]                      # 442 226 118 190
BMAX = 442                                           # uniform gather band length
HEAD = 512                                           # scratch head/tail guard
SOFF = [HEAD + P * sum(LHW[:i]) for i in range(NLVL)]  # level section offsets
SCRN1 = HEAD + P * NPOS + HEAD                       # per-tile scratch elems

F16 = mybir.dt.float16
F32 = mybir.dt.float32
I32 = mybir.dt.int32

PSUM_CHUNK = 1024
MM_CHUNK = 512


def _chunks(total, step):
    return [(o, min(step, total - o)) for o in range(0, total, step)]


def build_nc(repeat=1, do_write=True, do_gather=True, do_blend=True, do_mm=True,
             do_copy=True, ps_bufs=4, copy_cycle="va"):
    """copy_cycle: string over {'v','a','p'} giving the round-robin of
    psum->sbuf copy engines (DVE, ACT, Pool)."""
    nc = bacc.Bacc("TRN2", target_bir_lowering=False, debug=False)

    f1t = nc.dram_tensor("f1t", [2, P, QPC], F16, kind="ExternalInput")
    f2t = nc.dram_tensor("f2t", [2, P, NPOS], F16, kind="ExternalInput")
    idxt = nc.dram_tensor("idxt", [P, NLVL * NT], I32, kind="ExternalInput")
    # stage-2 blend per-query scalars, 2 per (lvl,tile)
    wgtt = nc.dram_tensor("wgtt", [P, NLVL * NT * 2], F32, kind="ExternalInput")
    # stage-1 blend weights with validity masks folded in, 90 per (lvl,tile)
    my0t = nc.dram_tensor("my0t", [P, NLVL * NT * 90], F16, kind="ExternalInput")
    my1t = nc.dram_tensor("my1t", [P, NLVL * NT * 90], F16, kind="ExternalInput")
    # per-tile [q, lvl*81] output blocks, host reorders
    outp = nc.dram_tensor("outp", [P, NT * NLVL * 81], F32, kind="ExternalOutput")

    copy_engines = {"v": nc.vector, "a": nc.scalar, "p": nc.gpsimd}

    with tile.TileContext(nc) as tc:
        with (
            tc.tile_pool(name="dram", bufs=1, space="DRAM") as dpool,
            tc.tile_pool(name="const", bufs=1) as cpool,
            tc.tile_pool(name="ck0", bufs=2) as ck0pool,
            tc.tile_pool(name="ckr", bufs=2) as ckrpool,
            tc.tile_pool(name="bands", bufs=4) as bpool,
            tc.tile_pool(name="blend", bufs=4) as blpool,
            tc.tile_pool(name="otile", bufs=3) as opool,
            tc.tile_pool(name="psum", bufs=ps_bufs, space="PSUM") as pspool,
        ):
            # ---- constants / persistent tiles ----
            scrt = [dpool.tile([SCRN1], F16, name=f"scrt{t}") for t in range(NT)]

            zguard = cpool.tile([1, HEAD], F16)
            nc.vector.memset(zguard[:], 0.0)
            # zero head+tail guards of every scratch so gathers never read
            # uninitialized HBM (inter-level band spill lands in written
            # neighbour sections; only the outer edges need zeroing).
            for t in range(NT):
                eng = nc.sync if t % 2 == 0 else nc.scalar
                eng.dma_start(scrt[t][0:HEAD].unsqueeze(0), zguard[0:1, :])
                eng.dma_start(
                    scrt[t][HEAD + P * NPOS : SCRN1].unsqueeze(0), zguard[0:1, :]
                )

            # f2 split by (level-0 / rest) x K-chunk so the first matmuls only
            # wait on their own section's load; loads split across SP and ACT
            # HWDGE queues so the preamble runs in parallel.
            f2a = [cpool.tile([P, L0N], F16, name=f"f2a{k}") for k in range(2)]
            f2b = [cpool.tile([P, RESTN], F16, name=f"f2b{k}") for k in range(2)]
            nc.sync.dma_start(f2a[0][:], f2t[0][:, 0:L0N])
            nc.scalar.dma_start(f2a[1][:], f2t[1][:, 0:L0N])
            nc.sync.dma_start(f2b[0][:], f2t[0][:, L0N:NPOS])
            nc.scalar.dma_start(f2b[1][:], f2t[1][:, L0N:NPOS])
            f1sb = cpool.tile([P, 2 * QPC], F16)
            nc.sync.dma_start(f1sb[:, 0:QPC], f1t[0])
            nc.scalar.dma_start(f1sb[:, QPC : 2 * QPC], f1t[1])
            idx_sb = cpool.tile([P, NLVL * NT], I32)
            nc.sync.dma_start(idx_sb[:], idxt[:])
            wgt_sb = cpool.tile([P, NLVL * NT * 2], F32)
            nc.scalar.dma_start(wgt_sb[:], wgtt[:])
            my0_sb = cpool.tile([P, NLVL * NT * 90], F16)
            nc.gpsimd.dma_start(my0_sb[:], my0t[:])
            my1_sb = cpool.tile([P, NLVL * NT * 90], F16)
            nc.gpsimd.dma_start(my1_sb[:], my1t[:])

            # ---- main loop over query tiles ----
            import contextlib

            rep_ctx = tc.For_i(0, repeat, 1) if repeat > 1 else contextlib.nullcontext()
            with rep_ctx:
                copy_rr = 0  # distribute psum->sbuf copies across engines
                for t in range(NT):
                    # === corr matmuls -> psum -> sbuf fp16 (per-level ck
                    # tiles) -> HBM scratch (one DMA per level on SP) ===
                    ck0 = ck0pool.tile([P, L0N], F16, name="ck0")
                    ckr = ckrpool.tile([P, RESTN], F16, name="ckr")
                    cks = {0: (ck0, 0), 1: (ckr, LOFF[1] - L0N),
                           2: (ckr, LOFF[2] - L0N), 3: (ckr, LOFF[3] - L0N)}
                    for l in range(NLVL):
                        hw = LHW[l]
                        ckt, ckoff = cks[l]
                        for coff, csz in _chunks(hw, PSUM_CHUNK):
                            ps = pspool.tile([P, PSUM_CHUNK], F32, name="cps")[:, :csz]
                            for k in range(2 if do_mm else 0):
                                fsec = f2a[k] if l == 0 else f2b[k]
                                soff0 = coff if l == 0 else LOFF[l] - L0N + coff
                                for soff, ssz in _chunks(csz, MM_CHUNK):
                                    nc.tensor.matmul(
                                        ps[:, soff : soff + ssz],
                                        f1sb[:, k * QPC + t * P : k * QPC + (t + 1) * P],
                                        fsec[:, soff0 + soff : soff0 + soff + ssz],
                                        start=(k == 0),
                                        stop=(k == 1),
                                    )
                            if not (do_mm and do_copy):
                                continue
                            eng = copy_engines[copy_cycle[copy_rr % len(copy_cycle)]]
                            ckv = ckt[:, ckoff + coff : ckoff + coff + csz]
                            if eng is nc.scalar:
                                eng.copy(ckv, ps)
                            else:
                                eng.tensor_copy(ckv, ps)
                            copy_rr += 1
                        if not (do_mm and do_copy and do_write):
                            continue
                        nc.sync.dma_start(
                            scrt[t][SOFF[l] : SOFF[l] + P * hw]
                            .rearrange("(p x) -> p x", x=hw),
                            ckt[:, ckoff : ckoff + hw],
                        )

                    # === 4 gathers per tile (one per level, HW-validated
                    # one-index-per-partition shape), then blend ===
                    if do_gather and do_write:
                        band = bpool.tile([P, NLVL * BMAX], F16, name="band")
                        for l in range(NLVL):
                            nc.gpsimd.indirect_dma_start(
                                out=band[:, l * BMAX : l * BMAX + BAND[l]],
                                out_offset=None,
                                in_=scrt[t][:].unsqueeze(1),
                                in_offset=bass.IndirectOffsetOnAxis(
                                    ap=idx_sb[:, t * NLVL + l : t * NLVL + l + 1],
                                    axis=0,
                                ),
                                element_offset=0,
                            )
                    ot = opool.tile([P, NLVL * 81], F32, name="ot") if do_blend else None
                    for l in range(NLVL if (do_gather and do_write and do_blend) else 0):
                        s_in = ST[l]
                        # window views into this level's band section
                        lo = l * BMAX
                        bw = band[:, lo : lo + 10 * s_in].rearrange(
                            "p (r s) -> p r s", s=s_in
                        )
                        g0 = bw[:, 0:10, 0:9]
                        g1 = bw[:, 0:10, 1:10]
                        c90 = (l * NT + t) * 90
                        m0 = my0_sb[:, c90 : c90 + 90].rearrange(
                            "p (r j) -> p r j", j=9
                        )
                        m1 = my1_sb[:, c90 : c90 + 90].rearrange(
                            "p (r j) -> p r j", j=9
                        )
                        t1 = blpool.tile([P, 90], F32, name="t1")
                        t1v = t1[:].rearrange("p (r j) -> p r j", j=9)
                        t2 = blpool.tile([P, 90], F32, name="t2")
                        t2v = t2[:].rearrange("p (r j) -> p r j", j=9)
                        # stage-1 blend along inner axis, masks folded in
                        nc.gpsimd.tensor_tensor(
                            out=t1v, in0=g0, in1=m0, op=mybir.AluOpType.mult
                        )
                        nc.gpsimd.tensor_tensor(
                            out=t2v, in0=g1, in1=m1, op=mybir.AluOpType.mult
                        )
                        nc.gpsimd.tensor_add(out=t1[:], in0=t1[:], in1=t2[:])
                        # stage-2 blend along outer axis, per-query scalars:
                        # o = t1[0:9]*(1-wx)  (ACT), then o = t1[1:10]*wx + o (DVE)
                        t1r = t1[:].rearrange("p (r j) -> p r j", j=9)
                        c2 = (l * NT + t) * 2
                        ov = ot[:, l * 81 : (l + 1) * 81].rearrange(
                            "p (a j) -> p a j", j=9
                        )
                        nc.scalar.mul(ov, t1r[:, 0:9, :], wgt_sb[:, c2 : c2 + 1])
                        nc.vector.scalar_tensor_tensor(
                            out=ov,
                            in0=t1r[:, 1:10, :],
                            scalar=wgt_sb[:, c2 + 1 : c2 + 2],
                            in1=ov,
                            op0=mybir.AluOpType.mult,
                            op1=mybir.AluOpType.add,
                        )
                    if do_blend and do_gather and do_write:
                        nc.sync.dma_start(
                            outp[:, t * NLVL * 81 : (t + 1) * NLVL * 81], ot[:]
                        )

    nc.compile()
    return nc


# ---------------- host side ----------------

def _pool2(x):
    n, c, h, w = x.shape
    return x.reshape(n, c, h // 2, 2, w // 2, 2).mean(axis=(3, 5))


def _host_prep(fmap1, fmap2, coords):
    fmap1 = np.asarray(fmap1, np.float32)
    fmap2 = np.asarray(fmap2, np.float32)
    coords = np.asarray(coords, np.float32)
    scale = np.float32(1.0 / np.sqrt(D))

    # pooled fmap2 levels, flattened in storage orientation, scaled
    levels = []
    cur = fmap2 * scale
    for l in range(NLVL):
        if XMAJ[l]:
            levels.append(
                np.ascontiguousarray(cur.transpose(0, 1, 3, 2)).reshape(B, D, LHW[l])
            )
        else:
            levels.append(cur.reshape(B, D, LHW[l]))
        if l < NLVL - 1:
            cur = _pool2(cur)
    f2cat = np.concatenate(levels, axis=2).astype(np.float16)  # [B, D, NPOS]
    f1h = fmap1.astype(np.float16)

    cx = coords[:, 0].reshape(-1)  # [B*H*W], query q = b*H*W + h*W + w
    cy = coords[:, 1].reshape(-1)
    nq = cx.shape[0]

    idx_all = np.zeros((NLVL, nq), np.int32)
    wgt_all = np.zeros((NLVL, nq, 2), np.float32)
    my0_all = np.zeros((NLVL, nq, 10, 9), np.float16)
    my1_all = np.zeros((NLVL, nq, 10, 9), np.float16)
    q_tile = (np.arange(nq) % P).astype(np.int64)  # partition within tile
    rr = np.arange(10)
    for l in range(NLVL):
        inv = np.float32(1.0 / (1 << l))
        x = cx * inv
        y = cy * inv
        x0 = np.floor(x)
        y0 = np.floor(y)
        wx = (x - x0).astype(np.float32)
        wy = (y - y0).astype(np.float32)
        x0c = np.clip(x0, -5, LW[l] + 4).astype(np.int64)
        y0c = np.clip(y0, -5, LH[l] + 4).astype(np.int64)
        vx = ((x0[:, None] + rr[None, :] - 4) >= 0) & (
            (x0[:, None] + rr[None, :] - 4) <= LW[l] - 1
        )  # [nq, 10] validity of x-tap x0-4+i
        vy = ((y0[:, None] + rr[None, :] - 4) >= 0) & (
            (y0[:, None] + rr[None, :] - 4) <= LH[l] - 1
        )
        if XMAJ[l]:
            # outer = x (weight wx), inner = y (weight wy)
            idx_all[l] = (
                SOFF[l] + q_tile * LHW[l] + (x0c - 4) * LH[l] + (y0c - 4)
            ).astype(np.int32)
            wgt_all[l, :, 0] = 1.0 - wx
            wgt_all[l, :, 1] = wx
            m0 = vx[:, :, None] & vy[:, None, 0:9]
            m1 = vx[:, :, None] & vy[:, None, 1:10]
            my0_all[l] = m0 * (1.0 - wy)[:, None, None]
            my1_all[l] = m1 * wy[:, None, None]
        else:
            # outer = y (weight wy), inner = x (weight wx)
            idx_all[l] = (
                SOFF[l] + q_tile * LHW[l] + (y0c - 4) * LW[l] + (x0c - 4)
            ).astype(np.int32)
            wgt_all[l, :, 0] = 1.0 - wy
            wgt_all[l, :, 1] = wy
            m0 = vy[:, :, None] & vx[:, None, 0:9]
            m1 = vy[:, :, None] & vx[:, None, 1:10]
            my0_all[l] = m0 * (1.0 - wx)[:, None, None]
            my1_all[l] = m1 * wx[:, None, None]

    def core_map(c):
        b = c // (NCORES // B)
        cl = c % (NCORES // B)
        sl = slice(c * QPC, (c + 1) * QPC)
        f1c = f1h.reshape(B, D, H * W)[b][:, cl * QPC : (cl + 1) * QPC]
        return {
            "f1t": np.ascontiguousarray(f1c.reshape(2, P, QPC)),
            "f2t": np.ascontiguousarray(f2cat[b].reshape(2, P, NPOS)),
            "idxt": np.ascontiguousarray(
                idx_all[:, sl].reshape(NLVL, NT, P).transpose(2, 1, 0).reshape(P, -1)
            ),
            "wgtt": np.ascontiguousarray(
                wgt_all[:, sl].reshape(NLVL, NT, P, 2)
                .transpose(2, 0, 1, 3)
                .reshape(P, -1)
            ),
            "my0t": np.ascontiguousarray(
                my0_all[:, sl].reshape(NLVL, NT, P, 90)
                .transpose(2, 0, 1, 3)
                .reshape(P, -1)
            ),
            "my1t": np.ascontiguousarray(
                my1_all[:, sl].reshape(NLVL, NT, P, 90)
                .transpose(2, 0, 1, 3)
                .reshape(P, -1)
            ),
        }

    return [core_map(c) for c in range(NCORES)]


def assemble(results):
    out = np.empty((B, NLVL * 81, H * W), np.float32)
    for c in range(NCORES):
        b = c // (NCORES // B)
        lo = (c % (NCORES // B)) * QPC
        # device layout [p, t, l, a] -> [l, a, t*128+p]
        r = np.asarray(results[c]["outp"], np.float32).reshape(P, NT, NLVL, 81)
        r = r.transpose(2, 3, 1, 0).reshape(NLVL, 81, QPC)
        for l in range(NLVL):
            blk = r[l]
            if not XMAJ[l]:
                # stored channel order is bi*9+a; reference wants 9a+bi
                blk = blk.reshape(9, 9, QPC).transpose(1, 0, 2).reshape(81, QPC)
            out[b, l * 81 : (l + 1) * 81, lo : lo + QPC] = blk
    return out.reshape(B, NLVL * 81, H, W)


_NC_CACHE = {}


def get_nc():
    if "nc" not in _NC_CACHE:
        _NC_CACHE["nc"] = build_nc()
    return _NC_CACHE["nc"]


def kernel(fmap1, fmap2, coords):
    in_maps = _host_prep(fmap1, fmap2, coords)
    nc = get_nc()
    res = run_bass_kernel_spmd(nc, in_maps, core_ids=list(range(NCORES)))
    return assemble(res.results)


# revision 6
# speedup vs baseline: 1.2677x; 1.2677x over previous
"""Trainium2 Bass kernel for nn_CSFlow (RAFT-style correlation pyramid lookup).

Math restructure (exact up to fp16 rounding of stored corr values):
  - corr(q, pos) = <fmap1[:, q], fmap2[:, pos]> / sqrt(D). Pooling the corr
    volume over (i, j) == pooling fmap2 (linearity), so each pyramid level is
    its own matmul against a pooled fmap2.
  - All 81 lookup offsets of one query share the same fractional bilinear
    weights (integer offsets), so the lookup = gather of a 10x10 integer
    window + separable 2-tap blends with per-query weights.
  - The 10x10 window at a per-query position is fetched from an HBM scratch
    copy of that query's corr map with ONE indirect-DMA descriptor per
    (query, level): a contiguous band of 9*S+10 elements (S = inner-axis
    size) starting at the window origin. The data between window rows inside
    the band is simply never read (strided view). Out-of-range taps are
    zeroed exactly via host-precomputed masks folded into the stage-1 blend
    weights; band reads that spill outside a query's map hit neighbouring
    maps / pre-zeroed guard rows, so they are finite and masked.
  - Levels 0-2 store maps x-major (inner = y, size H_l >= 12); level 3
    stores y-major (inner = x, size 20) because H_3 = 6 < 10 would make the
    window view overlap. The host permutes level-3 output channels back.

Engine layout (per query tile): PE runs the corr matmuls back-to-back into
4 psum bufs; psum->sbuf fp16 copies are split across DVE/ACT/Pool; all
scratch-write DMA triggers live on the SP sequencer (cheapest trigger, no
compute to displace); gathers are Pool SWDGE; blends on DVE (stage 1) and
ACT+DVE (stage 2). Output tiles DMA out per tile in query-major layout;
the host does the [q, lvl*81] -> [lvl*81, q] reorder in assemble().

Sharding: 8 cores x 1920 queries (B*H*W = 15360 split contiguously; cores
0-3 handle batch 0, cores 4-7 batch 1). kernel() takes full inputs and
returns the full output; everything device-side runs SPMD on 8 cores.
"""

import numpy as np

import concourse.bass as bass
import concourse.mybir as mybir
import concourse.tile as tile
from concourse import bacc
from concourse.bass_utils import run_bass_kernel_spmd

# problem shape (hardcoded per harness contract)
B, D, H, W = 2, 256, 48, 160
NCORES = 8
QPC = (B * H * W) // NCORES      # 1920 queries per core
P = 128                          # queries per tile (partitions)
NT = QPC // P                    # 15 tiles per core
NLVL = 4
LH = [48, 24, 12, 6]
LW = [160, 80, 40, 20]
LHW = [LH[i] * LW[i] for i in range(NLVL)]           # 7680 1920 480 120
LOFF = [0, 7680, 9600, 10080]                        # col offset in f2 concat
NPOS = 10200
L0N = LHW[0]                                         # 7680
RESTN = NPOS - L0N                                   # 2520
XMAJ = [True, True, True, False]                     # storage orientation
ST = [48, 24, 12, 20]                                # inner-axis size
BAND = [9 * s + 10 for s in ST]                      # 442 226 118 190
BMAX = 442                                           # uniform gather band length
HEAD = 512                                           # scratch head/tail guard
S13 = HEAD + P * L0N                                 # merged L1-3 section base
F13OFF = [None, 0, 1920, 2400]                       # level offset in 2520 row
SCRN1 = HEAD + P * NPOS + HEAD                       # per-tile scratch elems

F16 = mybir.dt.float16
F32 = mybir.dt.float32
I32 = mybir.dt.int32

PSUM_CHUNK = 1024
MM_CHUNK = 512


def _chunks(total, step):
    return [(o, min(step, total - o)) for o in range(0, total, step)]


def build_nc(repeat=1, do_write=True, do_gather=True, do_blend=True, do_mm=True,
             do_copy=True, ps_bufs=4, copy_cycle="va"):
    """copy_cycle: string over {'v','a','p'} giving the round-robin of
    psum->sbuf copy engines (DVE, ACT, Pool)."""
    nc = bacc.Bacc("TRN2", target_bir_lowering=False, debug=False)

    f1t = nc.dram_tensor("f1t", [2, P, QPC], F16, kind="ExternalInput")
    f2t = nc.dram_tensor("f2t", [2, P, NPOS], F16, kind="ExternalInput")
    idxt = nc.dram_tensor("idxt", [P, NLVL * NT], I32, kind="ExternalInput")
    # stage-2 blend per-query scalars, 2 per (lvl,tile)
    wgtt = nc.dram_tensor("wgtt", [P, NLVL * NT * 2], F32, kind="ExternalInput")
    # stage-1 blend weights with validity masks folded in, 90 per (lvl,tile)
    my0t = nc.dram_tensor("my0t", [P, NLVL * NT * 90], F16, kind="ExternalInput")
    my1t = nc.dram_tensor("my1t", [P, NLVL * NT * 90], F16, kind="ExternalInput")
    # per-tile [q, lvl*81] output blocks, host reorders
    outp = nc.dram_tensor("outp", [P, NT * NLVL * 81], F32, kind="ExternalOutput")

    copy_engines = {"v": nc.vector, "a": nc.scalar, "p": nc.gpsimd}

    with tile.TileContext(nc) as tc:
        with (
            tc.tile_pool(name="dram", bufs=1, space="DRAM") as dpool,
            tc.tile_pool(name="const", bufs=1) as cpool,
            tc.tile_pool(name="ck0", bufs=2) as ck0pool,
            tc.tile_pool(name="ckr", bufs=2) as ckrpool,
            tc.tile_pool(name="bands", bufs=4) as bpool,
            tc.tile_pool(name="blend", bufs=4) as blpool,
            tc.tile_pool(name="otile", bufs=3) as opool,
            tc.tile_pool(name="psum", bufs=ps_bufs, space="PSUM") as pspool,
        ):
            # ---- constants / persistent tiles ----
            scrt = [dpool.tile([SCRN1], F16, name=f"scrt{t}") for t in range(NT)]

            zguard = cpool.tile([1, HEAD], F16)
            nc.vector.memset(zguard[:], 0.0)
            # zero head+tail guards of every scratch so gathers never read
            # uninitialized HBM (inter-level band spill lands in written
            # neighbour sections; only the outer edges need zeroing).
            for t in range(NT):
                eng = nc.sync if t % 2 == 0 else nc.scalar
                eng.dma_start(scrt[t][0:HEAD].unsqueeze(0), zguard[0:1, :])
                eng.dma_start(
                    scrt[t][HEAD + P * NPOS : SCRN1].unsqueeze(0), zguard[0:1, :]
                )

            # f2 split by (level-0 / rest) x K-chunk so the first matmuls only
            # wait on their own section's load; loads split across SP and ACT
            # HWDGE queues so the preamble runs in parallel.
            f2a = [cpool.tile([P, L0N], F16, name=f"f2a{k}") for k in range(2)]
            f2b = [cpool.tile([P, RESTN], F16, name=f"f2b{k}") for k in range(2)]
            nc.sync.dma_start(f2a[0][:], f2t[0][:, 0:L0N])
            nc.scalar.dma_start(f2a[1][:], f2t[1][:, 0:L0N])
            nc.sync.dma_start(f2b[0][:], f2t[0][:, L0N:NPOS])
            nc.scalar.dma_start(f2b[1][:], f2t[1][:, L0N:NPOS])
            f1sb = cpool.tile([P, 2 * QPC], F16)
            nc.sync.dma_start(f1sb[:, 0:QPC], f1t[0])
            nc.scalar.dma_start(f1sb[:, QPC : 2 * QPC], f1t[1])
            idx_sb = cpool.tile([P, NLVL * NT], I32)
            nc.sync.dma_start(idx_sb[:], idxt[:])
            wgt_sb = cpool.tile([P, NLVL * NT * 2], F32)
            nc.scalar.dma_start(wgt_sb[:], wgtt[:])
            my0_sb = cpool.tile([P, NLVL * NT * 90], F16)
            nc.gpsimd.dma_start(my0_sb[:], my0t[:])
            my1_sb = cpool.tile([P, NLVL * NT * 90], F16)
            nc.gpsimd.dma_start(my1_sb[:], my1t[:])

            # ---- main loop over query tiles ----
            import contextlib

            rep_ctx = tc.For_i(0, repeat, 1) if repeat > 1 else contextlib.nullcontext()
            with rep_ctx:
                copy_rr = 0  # distribute psum->sbuf copies across engines
                for t in range(NT):
                    # === corr matmuls -> psum -> sbuf fp16 (per-level ck
                    # tiles) -> HBM scratch (one DMA per level on SP) ===
                    ck0 = ck0pool.tile([P, L0N], F16, name="ck0")
                    ckr = ckrpool.tile([P, RESTN], F16, name="ckr")
                    cks = {0: (ck0, 0), 1: (ckr, LOFF[1] - L0N),
                           2: (ckr, LOFF[2] - L0N), 3: (ckr, LOFF[3] - L0N)}
                    for l in range(NLVL):
                        hw = LHW[l]
                        ckt, ckoff = cks[l]
                        for coff, csz in _chunks(hw, PSUM_CHUNK):
                            ps = pspool.tile([P, PSUM_CHUNK], F32, name="cps")[:, :csz]
                            for k in range(2 if do_mm else 0):
                                fsec = f2a[k] if l == 0 else f2b[k]
                                soff0 = coff if l == 0 else LOFF[l] - L0N + coff
                                for soff, ssz in _chunks(csz, MM_CHUNK):
                                    nc.tensor.matmul(
                                        ps[:, soff : soff + ssz],
                                        f1sb[:, k * QPC + t * P : k * QPC + (t + 1) * P],
                                        fsec[:, soff0 + soff : soff0 + soff + ssz],
                                        start=(k == 0),
                                        stop=(k == 1),
                                    )
                            if not (do_mm and do_copy):
                                continue
                            eng = copy_engines[copy_cycle[copy_rr % len(copy_cycle)]]
                            ckv = ckt[:, ckoff + coff : ckoff + coff + csz]
                            if eng is nc.scalar:
                                eng.copy(ckv, ps)
                            else:
                                eng.tensor_copy(ckv, ps)
                            copy_rr += 1
                        if not (do_mm and do_copy and do_write):
                            continue
                        if l == 0:
                            nc.sync.dma_start(
                                scrt[t][HEAD : HEAD + P * L0N]
                                .rearrange("(p x) -> p x", x=L0N),
                                ckt[:, 0:L0N],
                            )
                        elif l == NLVL - 1:
                            nc.sync.dma_start(
                                scrt[t][S13 : S13 + P * RESTN]
                                .rearrange("(p x) -> p x", x=RESTN),
                                ckr[:, 0:RESTN],
                            )

                    # === 4 gathers per tile (one per level, HW-validated
                    # one-index-per-partition shape), then blend ===
                    if do_gather and do_write:
                        band = bpool.tile([P, NLVL * BMAX], F16, name="band")
                        for l in range(NLVL):
                            nc.gpsimd.indirect_dma_start(
                                out=band[:, l * BMAX : l * BMAX + BAND[l]],
                                out_offset=None,
                                in_=scrt[t][:].unsqueeze(1),
                                in_offset=bass.IndirectOffsetOnAxis(
                                    ap=idx_sb[:, t * NLVL + l : t * NLVL + l + 1],
                                    axis=0,
                                ),
                                element_offset=0,
                            )
                    ot = opool.tile([P, NLVL * 81], F32, name="ot") if do_blend else None
                    for l in range(NLVL if (do_gather and do_write and do_blend) else 0):
                        s_in = ST[l]
                        # window views into this level's band section
                        lo = l * BMAX
                        bw = band[:, lo : lo + 10 * s_in].rearrange(
                            "p (r s) -> p r s", s=s_in
                        )
                        g0 = bw[:, 0:10, 0:9]
                        g1 = bw[:, 0:10, 1:10]
                        c90 = (l * NT + t) * 90
                        m0 = my0_sb[:, c90 : c90 + 90].rearrange(
                            "p (r j) -> p r j", j=9
                        )
                        m1 = my1_sb[:, c90 : c90 + 90].rearrange(
                            "p (r j) -> p r j", j=9
                        )
                        t1 = blpool.tile([P, 90], F32, name="t1")
                        t1v = t1[:].rearrange("p (r j) -> p r j", j=9)
                        t2 = blpool.tile([P, 90], F32, name="t2")
                        t2v = t2[:].rearrange("p (r j) -> p r j", j=9)
                        # stage-1 blend along inner axis, masks folded in
                        nc.gpsimd.tensor_tensor(
                            out=t1v, in0=g0, in1=m0, op=mybir.AluOpType.mult
                        )
                        nc.gpsimd.tensor_tensor(
                            out=t2v, in0=g1, in1=m1, op=mybir.AluOpType.mult
                        )
                        nc.gpsimd.tensor_add(out=t1[:], in0=t1[:], in1=t2[:])
                        # stage-2 blend along outer axis, per-query scalars:
                        # o = t1[0:9]*(1-wx)  (ACT), then o = t1[1:10]*wx + o (DVE)
                        t1r = t1[:].rearrange("p (r j) -> p r j", j=9)
                        c2 = (l * NT + t) * 2
                        ov = ot[:, l * 81 : (l + 1) * 81].rearrange(
                            "p (a j) -> p a j", j=9
                        )
                        nc.scalar.mul(ov, t1r[:, 0:9, :], wgt_sb[:, c2 : c2 + 1])
                        nc.vector.scalar_tensor_tensor(
                            out=ov,
                            in0=t1r[:, 1:10, :],
                            scalar=wgt_sb[:, c2 + 1 : c2 + 2],
                            in1=ov,
                            op0=mybir.AluOpType.mult,
                            op1=mybir.AluOpType.add,
                        )
                    if do_blend and do_gather and do_write:
                        nc.sync.dma_start(
                            outp[:, t * NLVL * 81 : (t + 1) * NLVL * 81], ot[:]
                        )

    nc.compile()
    return nc


# ---------------- host side ----------------

def _pool2(x):
    n, c, h, w = x.shape
    return x.reshape(n, c, h // 2, 2, w // 2, 2).mean(axis=(3, 5))


def _host_prep(fmap1, fmap2, coords):
    fmap1 = np.asarray(fmap1, np.float32)
    fmap2 = np.asarray(fmap2, np.float32)
    coords = np.asarray(coords, np.float32)
    scale = np.float32(1.0 / np.sqrt(D))

    # pooled fmap2 levels, flattened in storage orientation, scaled
    levels = []
    cur = fmap2 * scale
    for l in range(NLVL):
        if XMAJ[l]:
            levels.append(
                np.ascontiguousarray(cur.transpose(0, 1, 3, 2)).reshape(B, D, LHW[l])
            )
        else:
            levels.append(cur.reshape(B, D, LHW[l]))
        if l < NLVL - 1:
            cur = _pool2(cur)
    f2cat = np.concatenate(levels, axis=2).astype(np.float16)  # [B, D, NPOS]
    f1h = fmap1.astype(np.float16)

    cx = coords[:, 0].reshape(-1)  # [B*H*W], query q = b*H*W + h*W + w
    cy = coords[:, 1].reshape(-1)
    nq = cx.shape[0]

    idx_all = np.zeros((NLVL, nq), np.int32)
    wgt_all = np.zeros((NLVL, nq, 2), np.float32)
    my0_all = np.zeros((NLVL, nq, 10, 9), np.float16)
    my1_all = np.zeros((NLVL, nq, 10, 9), np.float16)
    q_tile = (np.arange(nq) % P).astype(np.int64)  # partition within tile
    rr = np.arange(10)
    for l in range(NLVL):
        inv = np.float32(1.0 / (1 << l))
        x = cx * inv
        y = cy * inv
        x0 = np.floor(x)
        y0 = np.floor(y)
        wx = (x - x0).astype(np.float32)
        wy = (y - y0).astype(np.float32)
        x0c = np.clip(x0, -5, LW[l] + 4).astype(np.int64)
        y0c = np.clip(y0, -5, LH[l] + 4).astype(np.int64)
        vx = ((x0[:, None] + rr[None, :] - 4) >= 0) & (
            (x0[:, None] + rr[None, :] - 4) <= LW[l] - 1
        )  # [nq, 10] validity of x-tap x0-4+i
        vy = ((y0[:, None] + rr[None, :] - 4) >= 0) & (
            (y0[:, None] + rr[None, :] - 4) <= LH[l] - 1
        )
        base = (HEAD + q_tile * LHW[0]) if l == 0 else (
            S13 + q_tile * (NPOS - L0N) + F13OFF[l])
        if XMAJ[l]:
            # outer = x (weight wx), inner = y (weight wy)
            idx_all[l] = (
                base + (x0c - 4) * LH[l] + (y0c - 4)
            ).astype(np.int32)
            wgt_all[l, :, 0] = 1.0 - wx
            wgt_all[l, :, 1] = wx
            m0 = vx[:, :, None] & vy[:, None, 0:9]
            m1 = vx[:, :, None] & vy[:, None, 1:10]
            my0_all[l] = m0 * (1.0 - wy)[:, None, None]
            my1_all[l] = m1 * wy[:, None, None]
        else:
            # outer = y (weight wy), inner = x (weight wx)
            idx_all[l] = (
                base + (y0c - 4) * LW[l] + (x0c - 4)
            ).astype(np.int32)
            wgt_all[l, :, 0] = 1.0 - wy
            wgt_all[l, :, 1] = wy
            m0 = vy[:, :, None] & vx[:, None, 0:9]
            m1 = vy[:, :, None] & vx[:, None, 1:10]
            my0_all[l] = m0 * (1.0 - wx)[:, None, None]
            my1_all[l] = m1 * wx[:, None, None]

    def core_map(c):
        b = c // (NCORES // B)
        cl = c % (NCORES // B)
        sl = slice(c * QPC, (c + 1) * QPC)
        f1c = f1h.reshape(B, D, H * W)[b][:, cl * QPC : (cl + 1) * QPC]
        return {
            "f1t": np.ascontiguousarray(f1c.reshape(2, P, QPC)),
            "f2t": np.ascontiguousarray(f2cat[b].reshape(2, P, NPOS)),
            "idxt": np.ascontiguousarray(
                idx_all[:, sl].reshape(NLVL, NT, P).transpose(2, 1, 0).reshape(P, -1)
            ),
            "wgtt": np.ascontiguousarray(
                wgt_all[:, sl].reshape(NLVL, NT, P, 2)
                .transpose(2, 0, 1, 3)
                .reshape(P, -1)
            ),
            "my0t": np.ascontiguousarray(
                my0_all[:, sl].reshape(NLVL, NT, P, 90)
                .transpose(2, 0, 1, 3)
                .reshape(P, -1)
            ),
            "my1t": np.ascontiguousarray(
                my1_all[:, sl].reshape(NLVL, NT, P, 90)
                .transpose(2, 0, 1, 3)
                .reshape(P, -1)
            ),
        }

    return [core_map(c) for c in range(NCORES)]


def assemble(results):
    out = np.empty((B, NLVL * 81, H * W), np.float32)
    for c in range(NCORES):
        b = c // (NCORES // B)
        lo = (c % (NCORES // B)) * QPC
        # device layout [p, t, l, a] -> [l, a, t*128+p]
        r = np.asarray(results[c]["outp"], np.float32).reshape(P, NT, NLVL, 81)
        r = r.transpose(2, 3, 1, 0).reshape(NLVL, 81, QPC)
        for l in range(NLVL):
            blk = r[l]
            if not XMAJ[l]:
                # stored channel order is bi*9+a; reference wants 9a+bi
                blk = blk.reshape(9, 9, QPC).transpose(1, 0, 2).reshape(81, QPC)
            out[b, l * 81 : (l + 1) * 81, lo : lo + QPC] = blk
    return out.reshape(B, NLVL * 81, H, W)


_NC_CACHE = {}


def get_nc():
    if "nc" not in _NC_CACHE:
        _NC_CACHE["nc"] = build_nc()
    return _NC_CACHE["nc"]


def kernel(fmap1, fmap2, coords):
    in_maps = _host_prep(fmap1, fmap2, coords)
    nc = get_nc()
    res = run_bass_kernel_spmd(nc, in_maps, core_ids=list(range(NCORES)))
    return assemble(res.results)


# revision 18
# speedup vs baseline: 1.4916x; 1.1766x over previous
"""Trainium2 Bass kernel for nn_CSFlow (RAFT-style correlation pyramid lookup).

Math restructure (exact up to fp16 rounding of stored corr values):
  - corr(q, pos) = <fmap1[:, q], fmap2[:, pos]> / sqrt(D). Pooling the corr
    volume over (i, j) == pooling fmap2 (linearity), so each pyramid level is
    its own matmul against a pooled fmap2.
  - All 81 lookup offsets of one query share the same fractional bilinear
    weights (integer offsets), so the lookup = gather of a 10x10 integer
    window + separable 2-tap blends with per-query weights.
  - The 10x10 window at a per-query position is fetched from an HBM scratch
    copy of that query's corr map with ONE indirect-DMA descriptor per
    (query, level): a contiguous band of 9*S+10 elements (S = inner-axis
    size) starting at the window origin. The data between window rows inside
    the band is simply never read (strided view). Out-of-range taps are
    zeroed exactly via host-precomputed masks folded into the stage-1 blend
    weights; band reads that spill outside a query's map hit neighbouring
    maps / pre-zeroed guard rows, so they are finite and masked.
  - Levels 0-2 store maps x-major (inner = y, size H_l >= 12); level 3
    stores y-major (inner = x, size 20) because H_3 = 6 < 10 would make the
    window view overlap. The host permutes level-3 output channels back.

Engine layout (per query tile): PE runs the corr matmuls back-to-back into
4 psum bufs; psum->sbuf fp16 copies are split across DVE/ACT/Pool; all
scratch-write DMA triggers live on the SP sequencer (cheapest trigger, no
compute to displace); gathers are Pool SWDGE; blends on DVE (stage 1) and
ACT+DVE (stage 2). Output tiles DMA out per tile in query-major layout;
the host does the [q, lvl*81] -> [lvl*81, q] reorder in assemble().

Sharding: 8 cores x 1920 queries (B*H*W = 15360 split contiguously; cores
0-3 handle batch 0, cores 4-7 batch 1). kernel() takes full inputs and
returns the full output; everything device-side runs SPMD on 8 cores.
"""

import numpy as np

import concourse.bass as bass
from concourse.ap import AP
import concourse.mybir as mybir
import concourse.tile as tile
from concourse import bacc
from concourse.bass_utils import run_bass_kernel_spmd

# problem shape (hardcoded per harness contract)
B, D, H, W = 2, 256, 48, 160
NCORES = 8
QPC = (B * H * W) // NCORES      # 1920 queries per core
P = 128                          # queries per tile (partitions)
NT = QPC // P                    # 15 tiles per core
NLVL = 4
LH = [48, 24, 12, 6]
LW = [160, 80, 40, 20]
LHW = [LH[i] * LW[i] for i in range(NLVL)]           # 7680 1920 480 120
XMAJ = [True, True, True, False]                     # storage orientation
# Level 0 is stored banded: per core, the y axis is shifted by the core's
# first row (roll = 12*(core%4)) into z = y - roll, stored as zi = z + 16
# over ZST=46 columns per x (zeros where y falls outside the image). Every
# query tile t then only ever reads z in [a-16, a+19), a = (t*128)//160 --
# a STATIC 35-wide window per tile, identical on all cores -- so the matmul
# and scratch write cover 160x35 = 5600 positions instead of 7680.
ZST = 46                                             # stored z per x column
W0 = 35                                              # per-tile z-window width
XB = 14                                              # x-block: 14*35=490 cols
F2L0N = 160 * ZST                                    # 7360 f2 level-0 cols
L0N = 160 * W0                                       # 5600 written L0 cols
RESTN = 2520                                         # L1-3 cols
NPOS = F2L0N + RESTN                                 # f2 concat width
ST = [W0, 24, 12, 20]                                # band inner-axis size
BAND = [9 * s + 10 for s in ST]                      # 325 226 118 190
BMAX = 442                                           # uniform band slot length
HEAD = 512                                           # scratch head/tail guard
S13 = HEAD + P * L0N                                 # merged L1-3 section base
F13OFF = [None, 0, 1920, 2400]                       # level offset in 2520 row
SCRN1 = HEAD + P * (L0N + RESTN) + HEAD              # per-tile scratch elems

F16 = mybir.dt.float16
F32 = mybir.dt.float32
I32 = mybir.dt.int32

PSUM_GROUP = 2048
PSUM_CHUNK = 1024
MM_CHUNK = 512


def _chunks(total, step):
    return [(o, min(step, total - o)) for o in range(0, total, step)]


def build_nc(repeat=1, do_write=True, do_gather=True, do_blend=True, do_mm=True,
             do_copy=True, ps_bufs=2, copy_cycle="va", fused_gather=False):
    """copy_cycle: string over {'v','a','p'} giving the round-robin of
    psum->sbuf copy engines (DVE, ACT, Pool)."""
    nc = bacc.Bacc("TRN2", target_bir_lowering=False, debug=False)

    f1t = nc.dram_tensor("f1t", [2, P, QPC], F16, kind="ExternalInput")
    f2t = nc.dram_tensor("f2t", [2, P, NPOS], F16, kind="ExternalInput")
    idxt = nc.dram_tensor("idxt", [P, NLVL * NT], I32, kind="ExternalInput")
    # stage-2 blend per-query scalars, 2 per (lvl,tile)
    wgtt = nc.dram_tensor("wgtt", [P, NLVL * NT * 2], F32, kind="ExternalInput")
    # stage-1 blend weights with validity masks folded in, interleaved
    # (m0[r,j], m1[r,j]) pairs: 180 per (lvl,tile)
    my01t = nc.dram_tensor("my01t", [P, NLVL * NT * 180], F16, kind="ExternalInput")
    # per-tile [q, lvl*81] output blocks, host reorders
    outp = nc.dram_tensor("outp", [P, NT * NLVL * 81], F32, kind="ExternalOutput")

    copy_engines = {"v": nc.vector, "a": nc.scalar, "p": nc.gpsimd}

    with tile.TileContext(nc) as tc:
        with (
            tc.tile_pool(name="dram", bufs=1, space="DRAM") as dpool,
            tc.tile_pool(name="const", bufs=1) as cpool,
            tc.tile_pool(name="ck0", bufs=2) as ck0pool,
            tc.tile_pool(name="ckr", bufs=2) as ckrpool,
            tc.tile_pool(name="bands", bufs=4) as bpool,
            tc.tile_pool(name="blend", bufs=4) as blpool,
            tc.tile_pool(name="otile", bufs=3) as opool,
            tc.tile_pool(name="psum", bufs=ps_bufs, space="PSUM") as pspool,
        ):
            # ---- constants / persistent tiles ----
            scrt = [dpool.tile([SCRN1], F16, name=f"scrt{t}") for t in range(NT)]

            zguard = cpool.tile([1, HEAD], F16)
            nc.vector.memset(zguard[:], 0.0)
            # zero head+tail guards of every scratch so gathers never read
            # uninitialized HBM (inter-level band spill lands in written
            # neighbour sections; only the outer edges need zeroing).
            for t in range(NT):
                eng = nc.sync if t % 2 == 0 else nc.scalar
                eng.dma_start(scrt[t][0:HEAD].unsqueeze(0), zguard[0:1, :])
                eng.dma_start(
                    scrt[t][S13 + P * RESTN : SCRN1].unsqueeze(0), zguard[0:1, :]
                )

            # f2 split by (level-0 / rest) x K-chunk so the first matmuls only
            # wait on their own section's load; loads split across SP and ACT
            # HWDGE queues so the preamble runs in parallel.
            f2a = [cpool.tile([P, F2L0N], F16, name=f"f2a{k}") for k in range(2)]
            f2b = [cpool.tile([P, RESTN], F16, name=f"f2b{k}") for k in range(2)]
            nc.sync.dma_start(f2a[0][:], f2t[0][:, 0:F2L0N])
            nc.scalar.dma_start(f2a[1][:], f2t[1][:, 0:F2L0N])
            nc.sync.dma_start(f2b[0][:], f2t[0][:, F2L0N:NPOS])
            nc.scalar.dma_start(f2b[1][:], f2t[1][:, F2L0N:NPOS])
            f1sb = cpool.tile([P, 2 * QPC], F16)
            nc.sync.dma_start(f1sb[:, 0:QPC], f1t[0])
            nc.scalar.dma_start(f1sb[:, QPC : 2 * QPC], f1t[1])
            idx_sb = cpool.tile([P, NLVL * NT], I32)
            nc.sync.dma_start(idx_sb[:], idxt[:])
            wgt_sb = cpool.tile([P, NLVL * NT * 2], F32)
            nc.scalar.dma_start(wgt_sb[:], wgtt[:])
            my01_sb = cpool.tile([P, NLVL * NT * 180], F16)
            nc.gpsimd.dma_start(my01_sb[:], my01t[:])

            # ---- main loop over query tiles ----
            import contextlib

            rep_ctx = tc.For_i(0, repeat, 1) if repeat > 1 else contextlib.nullcontext()
            with rep_ctx:
                copy_rr = 0  # distribute psum->sbuf copies across engines
                for t in range(NT):
                    # === corr matmuls -> psum -> sbuf fp16 (ck0/ckr tiles)
                    # -> HBM scratch (2 DMAs per tile on SP). Column space is
                    # [L0: 7680 | L1-3: 2520]; matmuls run k-OUTER within a
                    # 2048-col psum group so the stationary f1 block is
                    # reloaded 2x per group instead of per 512-col matmul. ===
                    ck0 = ck0pool.tile([P, L0N], F16, name="ck0")
                    ckr = ckrpool.tile([P, RESTN], F16, name="ckr")
                    a0 = (t * P) // 160  # tile's static z-window base
                    f1v = [
                        f1sb[:, k * QPC + t * P : k * QPC + (t + 1) * P]
                        for k in range(2)
                    ]

                    def copy_to(ckv, psv):
                        nonlocal copy_rr
                        eng = copy_engines[copy_cycle[copy_rr % len(copy_cycle)]]
                        if eng is nc.scalar:
                            eng.copy(ckv, psv)
                        else:
                            eng.tensor_copy(ckv, psv)
                        copy_rr += 1

                    # L0: 12 x-blocks of [XB,35] cols, 4 per 2048-psum group,
                    # each block bank-aligned at 512 inside the group.
                    xblks = _chunks(160, XB)
                    for g in range(0, len(xblks), 4):
                        blks = xblks[g : g + 4]
                        ps = pspool.tile([P, PSUM_GROUP], F32, name="cps")
                        for k in range(2 if do_mm else 0):
                            for bi, (xs, xn) in enumerate(blks):
                                rhs = f2a[k][:].rearrange(
                                    "p (x z) -> p x z", z=ZST
                                )[:, xs : xs + xn, a0 : a0 + W0]
                                nc.tensor.matmul(
                                    ps[:, bi * 512 : bi * 512 + xn * W0],
                                    f1v[k],
                                    rhs,
                                    start=(k == 0),
                                    stop=(k == 1),
                                )
                        if not (do_mm and do_copy):
                            continue
                        # copy full blocks as one strided op, remainder alone
                        nfull = sum(1 for xs, xn in blks if xn == XB)
                        ck_lo = blks[0][0] * W0
                        if nfull:
                            psv = ps[:].rearrange("p (b c) -> p b c", c=512)[
                                :, 0:nfull, 0 : XB * W0
                            ]
                            ckv = ck0[:, ck_lo : ck_lo + nfull * XB * W0].rearrange(
                                "p (b c) -> p b c", c=XB * W0
                            )
                            copy_to(ckv, psv)
                        for bi in range(nfull, len(blks)):
                            xs, xn = blks[bi]
                            copy_to(
                                ck0[:, xs * W0 : (xs + xn) * W0],
                                ps[:, bi * 512 : bi * 512 + xn * W0],
                            )
                    # L1-3: contiguous 2520 cols in f2b/ckr
                    for goff, gsz in _chunks(RESTN, PSUM_GROUP):
                        ps = pspool.tile([P, PSUM_GROUP], F32, name="cps")[:, :gsz]
                        for k in range(2 if do_mm else 0):
                            for soff, ssz in _chunks(gsz, MM_CHUNK):
                                nc.tensor.matmul(
                                    ps[:, soff : soff + ssz],
                                    f1v[k],
                                    f2b[k][:, goff + soff : goff + soff + ssz],
                                    start=(k == 0),
                                    stop=(k == 1),
                                )
                        if not (do_mm and do_copy):
                            continue
                        for coff, csz in _chunks(gsz, PSUM_CHUNK):
                            copy_to(
                                ckr[:, goff + coff : goff + coff + csz],
                                ps[:, coff : coff + csz],
                            )
                    if do_mm and do_copy and do_write:
                        nc.sync.dma_start(
                            scrt[t][HEAD : HEAD + P * L0N]
                            .rearrange("(p x) -> p x", x=L0N),
                            ck0[:, 0:L0N],
                        )
                        nc.sync.dma_start(
                            scrt[t][S13 : S13 + P * RESTN]
                            .rearrange("(p x) -> p x", x=RESTN),
                            ckr[:, 0:RESTN],
                        )

                    # === one 4-level gather per tile (4 bands per query at
                    # uniform BMAX stride; over-read past short bands lands in
                    # written neighbour sections / tail guard), then blend ===
                    if do_gather and do_write:
                        band = bpool.tile([P, NLVL * BMAX], F16, name="band")
                        if fused_gather:
                            nc.gpsimd.indirect_dma_start(
                                out=band[:].rearrange("p (l b) -> p l b", b=BMAX),
                                out_offset=None,
                                in_=scrt[t][:].unsqueeze(1),
                                in_offset=bass.IndirectOffsetOnAxis(
                                    ap=idx_sb[:, t * NLVL : (t + 1) * NLVL],
                                    axis=0,
                                ),
                                element_offset=0,
                            )
                        else:
                            for l in range(NLVL):
                                nc.gpsimd.indirect_dma_start(
                                    out=band[:, l * BMAX : l * BMAX + BAND[l]],
                                    out_offset=None,
                                    in_=scrt[t][:].unsqueeze(1),
                                    in_offset=bass.IndirectOffsetOnAxis(
                                        ap=idx_sb[:, t * NLVL + l : t * NLVL + l + 1],
                                        axis=0,
                                    ),
                                    element_offset=0,
                                )
                    ot = opool.tile([P, NLVL * 81], F32, name="ot") if do_blend else None
                    for l in range(NLVL if (do_gather and do_write and do_blend) else 0):
                        s_in = ST[l]
                        # stage-1: paired mult over overlapping band taps
                        # (g[j], g[j+1]) x interleaved (m0[j], m1[j]) in ONE
                        # op, then a strided pair-sum. Levels alternate
                        # Pool/DVE so the 4-op chains run concurrently.
                        lo = l * BMAX
                        b3 = band[:, lo : lo + 10 * s_in].rearrange(
                            "p (r s) -> p r s", s=s_in
                        )[:, 0:10, 0:9]
                        bp = AP(b3.tensor, b3.offset, list(b3.ap) + [(1, 2)])
                        c180 = (l * NT + t) * 180
                        m01 = my01_sb[:, c180 : c180 + 180].rearrange(
                            "p (r j e) -> p r j e", j=9, e=2
                        )
                        e1 = nc.gpsimd if l % 2 == 0 else nc.vector
                        e2 = nc.vector
                        t12 = blpool.tile([P, 180], F32, name="t12")
                        t12v = t12[:].rearrange("p (r j e) -> p r j e", j=9, e=2)
                        e1.tensor_tensor(
                            out=t12v, in0=bp, in1=m01, op=mybir.AluOpType.mult
                        )
                        t1 = blpool.tile([P, 90], F32, name="t1")
                        t1v = t1[:].rearrange("p (r j) -> p r j", j=9)
                        tse = t12[:].rearrange("p (r j e) -> p r j e", j=9, e=2)
                        e1.tensor_tensor(
                            out=t1v, in0=tse[:, :, :, 0], in1=tse[:, :, :, 1],
                            op=mybir.AluOpType.add,
                        )
                        # stage-2 blend along outer axis, per-query scalars:
                        # o = t1[0:9]*(1-wx)  (ACT), then o = t1[1:10]*wx + o
                        t1r = t1[:].rearrange("p (r j) -> p r j", j=9)
                        c2 = (l * NT + t) * 2
                        ov = ot[:, l * 81 : (l + 1) * 81].rearrange(
                            "p (a j) -> p a j", j=9
                        )
                        nc.scalar.mul(ov, t1r[:, 0:9, :], wgt_sb[:, c2 : c2 + 1])
                        e2.scalar_tensor_tensor(
                            out=ov,
                            in0=t1r[:, 1:10, :],
                            scalar=wgt_sb[:, c2 + 1 : c2 + 2],
                            in1=ov,
                            op0=mybir.AluOpType.mult,
                            op1=mybir.AluOpType.add,
                        )
                    if do_blend and do_gather and do_write:
                        nc.sync.dma_start(
                            outp[:, t * NLVL * 81 : (t + 1) * NLVL * 81], ot[:]
                        )

    nc.compile()
    return nc


# ---------------- host side ----------------

def _pool2(x):
    n, c, h, w = x.shape
    return x.reshape(n, c, h // 2, 2, w // 2, 2).mean(axis=(3, 5))


def _host_prep(fmap1, fmap2, coords):
    fmap1 = np.asarray(fmap1, np.float32)
    fmap2 = np.asarray(fmap2, np.float32)
    coords = np.asarray(coords, np.float32)
    scale = np.float32(1.0 / np.sqrt(D))

    # pooled fmap2 levels, flattened in storage orientation, scaled
    levels = []
    cur = fmap2 * scale
    for l in range(NLVL):
        if XMAJ[l]:
            levels.append(
                np.ascontiguousarray(cur.transpose(0, 1, 3, 2)).reshape(B, D, LHW[l])
            )
        else:
            levels.append(cur.reshape(B, D, LHW[l]))
        if l < NLVL - 1:
            cur = _pool2(cur)
    l0full = (fmap2 * scale).astype(np.float32)  # [B, D, 48, 160]
    rest = np.concatenate(levels[1:], axis=2).astype(np.float16)  # [B, D, 2520]

    # per-roll z-shifted level-0 storage: zi = y - roll + 16 over ZST columns,
    # x-major, zeros where y falls outside the image
    f2l0z = {}
    for roll in (0, 12, 24, 36):
        zimg = np.zeros((B, D, ZST, 160), np.float16)
        ylo, yhi = max(0, roll - 16), min(H, roll + ZST - 16)
        zimg[:, :, ylo - roll + 16 : yhi - roll + 16, :] = l0full[:, :, ylo:yhi, :]
        f2l0z[roll] = np.ascontiguousarray(
            zimg.transpose(0, 1, 3, 2)
        ).reshape(B, D, F2L0N)
    f1h = fmap1.astype(np.float16)

    cx = coords[:, 0].reshape(-1)  # [B*H*W], query q = b*H*W + h*W + w
    cy = coords[:, 1].reshape(-1)
    nq = cx.shape[0]

    idx_all = np.zeros((NLVL, nq), np.int32)
    wgt_all = np.zeros((NLVL, nq, 2), np.float32)
    my0_all = np.zeros((NLVL, nq, 10, 9), np.float16)
    my1_all = np.zeros((NLVL, nq, 10, 9), np.float16)
    q_tile = (np.arange(nq) % P).astype(np.int64)  # partition within tile
    rr = np.arange(10)
    for l in range(NLVL):
        inv = np.float32(1.0 / (1 << l))
        x = cx * inv
        y = cy * inv
        x0 = np.floor(x)
        y0 = np.floor(y)
        wx = (x - x0).astype(np.float32)
        wy = (y - y0).astype(np.float32)
        x0c = np.clip(x0, -5, LW[l] + 4).astype(np.int64)
        y0c = np.clip(y0, -5, LH[l] + 4).astype(np.int64)
        vx = ((x0[:, None] + rr[None, :] - 4) >= 0) & (
            (x0[:, None] + rr[None, :] - 4) <= LW[l] - 1
        )  # [nq, 10] validity of x-tap x0-4+i
        vy = ((y0[:, None] + rr[None, :] - 4) >= 0) & (
            (y0[:, None] + rr[None, :] - 4) <= LH[l] - 1
        )
        if XMAJ[l]:
            # outer = x (weight wx), inner = y (weight wy)
            if l == 0:
                # banded storage: inner axis is zi_local in the tile's static
                # 35-wide z-window [a-16, a+19), z = y - roll
                qg = np.arange(nq)
                roll_q = 12 * ((qg // QPC) % 4)
                a_q = ((qg % QPC) // P * P) // 160
                idx_all[l] = (
                    HEAD + q_tile * L0N + (x0c - 4) * W0
                    + (y0c - 4) - roll_q - a_q + 16
                ).astype(np.int32)
            else:
                idx_all[l] = (
                    S13 + q_tile * RESTN + F13OFF[l] + (x0c - 4) * LH[l] + (y0c - 4)
                ).astype(np.int32)
            wgt_all[l, :, 0] = 1.0 - wx
            wgt_all[l, :, 1] = wx
            m0 = vx[:, :, None] & vy[:, None, 0:9]
            m1 = vx[:, :, None] & vy[:, None, 1:10]
            my0_all[l] = m0 * (1.0 - wy)[:, None, None]
            my1_all[l] = m1 * wy[:, None, None]
        else:
            # outer = y (weight wy), inner = x (weight wx)
            idx_all[l] = (
                S13 + q_tile * RESTN + F13OFF[l] + (y0c - 4) * LW[l] + (x0c - 4)
            ).astype(np.int32)
            wgt_all[l, :, 0] = 1.0 - wy
            wgt_all[l, :, 1] = wy
            m0 = vy[:, :, None] & vx[:, None, 0:9]
            m1 = vy[:, :, None] & vx[:, None, 1:10]
            my0_all[l] = m0 * (1.0 - wx)[:, None, None]
            my1_all[l] = m1 * wx[:, None, None]

    def core_map(c):
        b = c // (NCORES // B)
        cl = c % (NCORES // B)
        sl = slice(c * QPC, (c + 1) * QPC)
        f1c = f1h.reshape(B, D, H * W)[b][:, cl * QPC : (cl + 1) * QPC]
        f2c = np.concatenate([f2l0z[12 * cl][b], rest[b]], axis=1)
        return {
            "f1t": np.ascontiguousarray(f1c.reshape(2, P, QPC)),
            "f2t": np.ascontiguousarray(f2c.reshape(2, P, NPOS)),
            "idxt": np.ascontiguousarray(
                idx_all[:, sl].reshape(NLVL, NT, P).transpose(2, 1, 0).reshape(P, -1)
            ),
            "wgtt": np.ascontiguousarray(
                wgt_all[:, sl].reshape(NLVL, NT, P, 2)
                .transpose(2, 0, 1, 3)
                .reshape(P, -1)
            ),
            "my01t": np.ascontiguousarray(
                np.stack(
                    [my0_all[:, sl], my1_all[:, sl]], axis=-1
                )  # [NLVL, QPC, 10, 9, 2]
                .reshape(NLVL, NT, P, 180)
                .transpose(2, 0, 1, 3)
                .reshape(P, -1)
            ),
        }

    return [core_map(c) for c in range(NCORES)]


def assemble(results):
    out = np.empty((B, NLVL * 81, H * W), np.float32)
    for c in range(NCORES):
        b = c // (NCORES // B)
        lo = (c % (NCORES // B)) * QPC
        # device layout [p, t, l, a] -> [l, a, t*128+p]
        r = np.asarray(results[c]["outp"], np.float32).reshape(P, NT, NLVL, 81)
        r = r.transpose(2, 3, 1, 0).reshape(NLVL, 81, QPC)
        for l in range(NLVL):
            blk = r[l]
            if not XMAJ[l]:
                # stored channel order is bi*9+a; reference wants 9a+bi
                blk = blk.reshape(9, 9, QPC).transpose(1, 0, 2).reshape(81, QPC)
            out[b, l * 81 : (l + 1) * 81, lo : lo + QPC] = blk
    return out.reshape(B, NLVL * 81, H, W)


_NC_CACHE = {}


def get_nc():
    if "nc" not in _NC_CACHE:
        _NC_CACHE["nc"] = build_nc()
    return _NC_CACHE["nc"]


def kernel(fmap1, fmap2, coords):
    in_maps = _host_prep(fmap1, fmap2, coords)
    nc = get_nc()
    res = run_bass_kernel_spmd(nc, in_maps, core_ids=list(range(NCORES)))
    return assemble(res.results)
